# revision 12
# baseline (speedup 1.0000x reference)
"""Trainium2 Bass kernel for a batched Kalman filter.

Math: the covariance/gain recursion of the Kalman filter is independent of the
measurements, and the initial covariance is identical for every batch element.
So the gain sequence K_t and transition A_t = (I - K_t H) F are batch-uniform
and computed once on the host (float64). The device evaluates, per batch
element b, the linear recurrence x_t = A_t x_{t-1} + K_t z_t, parallelized
over time in chunks of CH=8 steps:

    X[k][(i,s), b] = sum_{j<=i,o} L[k,i,j][s,o] zt[(j,o), b]
                   + sum_{s'}    G[k,i][s,s']  xt[s', b]

All operands are fp16 (PSUM accumulates fp32); the host pre-transposes the
measurements so no on-chip transposes are needed, and outputs leave transposed
as fp16 for host reassembly. Each chunk produces two PSUM tiles of 4 steps
(M = 4*32 = 128):

  A-tile (steps 0-3): ONE matmul — weights [la; ga] (K = 4*16 z-rows + 32
  state rows = 96) against the concatenated stream zg = [z(j=0..3) | x_entry].
  B-tile (steps 4-7, permuted ORDER_B): weights [lb(j=0..3); gb] (K=96)
  against zg, accumulated with lb(j=4..7) (K=64) against zhi = z(j=4..7).

The entry state x_entry lives INSIDE the zg tile at partitions 64:96: slot 0
is DMA'd (x0), slots k+1 are written by a narrow Activation-engine cast of
psB rows 64:96 — ORDER_B puts chunk-final step 7 exactly there, so the carry
is lane-aligned. 24 matmuls total.

The Riccati recursion converges within two chunks, so chunks k >= 2 share one
weight set (verified: output error identical to exact weights). Uploads and
the per-chunk output DMAs are spread across the Sync/Activation HWDGE queues
plus the GpSimd SWDGE queue, since each queue moves only ~90 GB/s serially.
"""

import numpy as np

import concourse.bass as bass
import concourse.mybir as mybir
import concourse.tile as tile
from concourse.bass_utils import run_bass_kernel_spmd

S_DIM = 32
O_DIM = 16
T = 64
CH = 8
NCH = T // CH
B = 2048
NCORES = 8
BS = B // NCORES  # 256
ORDER_B = (4, 5, 7, 6)  # step 7 at rows 64:96 -> lane-aligned carry cast
KSETS = 3  # distinct weight sets: chunk 0, chunk 1, steady state (k >= 2)

F32 = mybir.dt.float32
F16 = mybir.dt.float16


def _host_gains(F, H, Q, R, P0):
    """Batch-uniform Kalman gain/transition sequences, in float64."""
    I = np.eye(S_DIM)
    P = P0
    A_list, K_list = [], []
    for _ in range(T):
        P_pred = F @ P @ F.T + Q
        S = H @ P_pred @ H.T + R
        K = P_pred @ H.T @ np.linalg.inv(S)
        A = (I - K @ H) @ F
        P = (I - K @ H) @ P_pred
        A_list.append(A)
        K_list.append(K)

    G = np.zeros((KSETS, CH, S_DIM, S_DIM))
    L = np.zeros((KSETS, CH, CH, S_DIM, O_DIM))
    for k in range(KSETS):
        for i in range(CH):
            t = CH * k + i
            G[k, i] = A_list[t] @ (G[k, i - 1] if i > 0 else I)
            for j in range(i):
                L[k, i, j] = A_list[t] @ L[k, i - 1, j]
            L[k, i, i] = K_list[t]
    return G, L


def _pack_weights(G, L):
    """wall [96, KSETS, 3, 128]: slot 0 = [la; ga], slot 1 = [lb(j<4); gb]
    (B-tile cols in ORDER_B), slot 2 rows 0:64 = lb(j=4..7)."""
    wall = np.zeros((96, KSETS, 3, 128))
    for k in range(KSETS):
        for ii in range(4):
            wall[64:96, k, 0, ii * 32:(ii + 1) * 32] = G[k, ii].T
            for j in range(ii + 1):
                wall[j * 16:(j + 1) * 16, k, 0, ii * 32:(ii + 1) * 32] = \
                    L[k, ii, j].T
        for idx, i in enumerate(ORDER_B):
            wall[64:96, k, 1, idx * 32:(idx + 1) * 32] = G[k, i].T
            for j in range(min(i + 1, 4)):
                wall[j * 16:(j + 1) * 16, k, 1, idx * 32:(idx + 1) * 32] = \
                    L[k, i, j].T
            for j in range(4, i + 1):
                wall[(j - 4) * 16:(j - 3) * 16, k, 2,
                     idx * 32:(idx + 1) * 32] = L[k, i, j].T
    return wall.astype(np.float16)


def build_nc(split_waits=True):
    nc = bass.Bass("TRN2", target_bir_lowering=False, debug=False,
                   num_devices=NCORES)

    zgz_d = nc.dram_tensor("zgz", (64, NCH, BS), F16, kind="ExternalInput")
    zhi_d = nc.dram_tensor("zhi", (64, NCH, BS), F16, kind="ExternalInput")
    x0t_d = nc.dram_tensor("x0t", (S_DIM, BS), F16, kind="ExternalInput")
    wall_d = nc.dram_tensor("wall", (96, KSETS, 3, 128), F16,
                            kind="ExternalInput")
    out_d = nc.dram_tensor("out", (NCH, 2, 128, BS), F16, kind="ExternalOutput")

    with tile.TileContext(nc) as tc:
        with (
            tc.tile_pool(name="const", bufs=1) as const,
            tc.tile_pool(name="outs", bufs=4) as s_p,
            tc.tile_pool(name="psa", bufs=3, space="PSUM") as ps_a,
            tc.tile_pool(name="psb", bufs=3, space="PSUM") as ps_b,
        ):
            zg = const.tile([96, NCH, BS], F16)   # rows 0:64 z(j<4), 64:96 x
            zhi = const.tile([64, NCH, BS], F16)
            wall = const.tile([96, KSETS, 3, 128], F16)

            nc.scalar.dma_start(wall[:, 0:1, :, :], wall_d[:, 0:1, :, :])
            nc.sync.dma_start(zg[0:64, 0:1, :], zgz_d[:, 0:1, :])
            nc.scalar.dma_start(zg[64:96, 0, :], x0t_d[:])
            nc.sync.dma_start(zhi[:, 0:1, :], zhi_d[:, 0:1, :])
            nc.scalar.dma_start(wall[:, 1:, :, :], wall_d[:, 1:, :, :])
            nc.sync.dma_start(zg[0:64, 1:, :], zgz_d[:, 1:, :])
            nc.sync.dma_start(zhi[:, 1:, :], zhi_d[:, 1:, :])

            out_eng = [nc.gpsimd, nc.scalar]

            for k in range(NCH):
                ki = min(k, 2)
                last = k == NCH - 1
                zg_k = zg[0:96, k, :]

                psA = ps_a.tile([128, BS], F32, name="psA")
                nc.tensor.matmul(psA[:], wall[0:96, ki, 0, :], zg_k,
                                 start=True, stop=True)
                psB = ps_b.tile([128, BS], F32, name="psB")
                nc.tensor.matmul(psB[:], wall[0:64, ki, 2, :],
                                 zhi[0:64, k, :], start=True, stop=False)
                nc.tensor.matmul(psB[:], wall[0:96, ki, 1, :], zg_k,
                                 start=False, stop=True)
                if not last:
                    # carry: step-7 rows land lane-aligned into zg slot k+1
                    nc.scalar.copy(zg[64:96, k + 1, :], psB[64:96, :])

                pair = s_p.tile([128, 2, BS], F16, name="pair")
                nc.vector.tensor_copy(pair[:, 0, :], psA[:])
                nc.vector.tensor_copy(pair[:, 1, :], psB[:])
                if not last:
                    out_eng[k % 2].dma_start(
                        out_d[k].rearrange("two p b -> p two b"), pair[:])
                else:
                    nc.sync.dma_start(out_d[k, 0], pair[:, 0, :])
                    nc.scalar.dma_start(out_d[k, 1], pair[:, 1, :])

    if split_waits:
        # the wait-splitting NoOps confuse CoreSim's race detector; the sim
        # path builds without them (identical semantics, redistributed waits)
        _split_matmul_waits(nc)
    return nc


def _split_matmul_waits(nc, max_waits=1):
    """Walrus lowers matmuls/DMAs through templates that support fewer
    sync-wait slots than Tile may emit. Move excess waits onto a NoOp
    inserted right before the offending instruction."""
    for f in nc.m.functions:
        for blk in f.blocks:
            insts = list(blk.instructions)
            out = []
            for inst in insts:
                si = inst.sync_info
                if si is not None and si.on_wait and len(si.on_wait) > max_waits:
                    waits = list(si.on_wait)
                    carry, keep = waits[:-max_waits], waits[-max_waits:]
                    for w in carry:
                        nop = mybir.InstNoOp(
                            name=nc.get_next_instruction_name(),
                            sync_info=mybir.SyncInfo(on_wait=[w], on_update=[]),
                            bass_nofuse=True,
                            engine=inst.engine,
                        )
                        out.append(nop)
                    inst.sync_info = mybir.SyncInfo(
                        on_wait=keep, on_update=list(si.on_update or [])
                    )
                out.append(inst)
            if len(out) != len(insts):
                blk.instructions = out


def _prep_inputs(state0, cov0, measurements, F, H, Q, R):
    """Host-side: gains, packing, measurement pre-transpose. Returns per-core
    input maps."""
    G, L = _host_gains(
        np.asarray(F, np.float64), np.asarray(H, np.float64),
        np.asarray(Q, np.float64), np.asarray(R, np.float64),
        np.asarray(cov0, np.float64)[0],
    )
    wall = _pack_weights(G, L)

    state0 = np.asarray(state0, np.float32)
    measurements = np.asarray(measurements, np.float32)

    in_maps = []
    for c in range(NCORES):
        z = measurements[c * BS:(c + 1) * BS]
        zt = np.ascontiguousarray(
            z.reshape(BS, NCH, CH, O_DIM).transpose(2, 3, 1, 0)
        ).reshape(CH * O_DIM, NCH, BS).astype(np.float16)
        in_maps.append({
            "zgz": np.ascontiguousarray(zt[0:64]),
            "zhi": np.ascontiguousarray(zt[64:128]),
            "x0t": np.ascontiguousarray(
                state0[c * BS:(c + 1) * BS].T).astype(np.float16),
            "wall": wall,
        })
    return in_maps


def _assemble(results):
    """Stitch per-core transposed fp16 outputs into (B, T, S) fp32."""
    out = np.empty((B, T, S_DIM), np.float32)
    for c in range(NCORES):
        arr = np.asarray(results[c]["out"], np.float32).reshape(NCH, 2, 4, 32, BS)
        xA = arr[:, 0].transpose(3, 0, 1, 2)                    # i = 0..3
        xB = arr[:, 1][:, (0, 1, 3, 2)].transpose(3, 0, 1, 2)   # i = 4..7
        out[c * BS:(c + 1) * BS] = np.concatenate(
            [xA, xB], axis=2).reshape(BS, T, S_DIM)
    return out


_CACHE = {}


def kernel(state0, cov0, measurements, F, H, Q, R, _trace=False):
    in_maps = _prep_inputs(state0, cov0, measurements, F, H, Q, R)

    if "nc" not in _CACHE:
        _CACHE["nc"] = build_nc()
    nc = _CACHE["nc"]

    res = run_bass_kernel_spmd(nc, in_maps, core_ids=list(range(NCORES)),
                               trace=_trace)
    out = _assemble(res.results)
    if _trace:
        kernel._last_result = res
    return out


# revision 13
# speedup vs baseline: 1.0876x; 1.0876x over previous
"""Trainium2 Bass kernel for a batched Kalman filter.

Math: the covariance/gain recursion of the Kalman filter is independent of the
measurements, and the initial covariance is identical for every batch element.
So the gain sequence K_t and transition A_t = (I - K_t H) F are batch-uniform
and computed once on the host (float64). The device evaluates, per batch
element b, the linear recurrence x_t = A_t x_{t-1} + K_t z_t, parallelized
over time in chunks of CH=8 steps:

    X[k][(i,s), b] = sum_{j<=i,o} L[k,i,j][s,o] zt[(j,o), b]
                   + sum_{s'}    G[k,i][s,s']  xt[s', b]

All operands are fp16 (PSUM accumulates fp32); the host pre-transposes the
measurements so no on-chip transposes are needed, and outputs leave transposed
as fp16 for host reassembly. Each chunk produces two PSUM tiles of 4 steps
(M = 4*32 = 128):

  A-tile (steps 0-3): ONE matmul — weights [la; ga] (K = 4*16 z-rows + 32
  state rows = 96) against the concatenated stream zg = [z(j=0..3) | x_entry].
  B-tile (steps 4-7, permuted ORDER_B): weights [lb(j=0..3); gb] (K=96)
  against zg, accumulated with lb(j=4..7) (K=64) against zhi = z(j=4..7).

The entry state x_entry lives INSIDE the zg tile at partitions 64:96: slot 0
is DMA'd (x0), slots k+1 are written by a narrow Activation-engine cast of
psB rows 64:96 — ORDER_B puts chunk-final step 7 exactly there, so the carry
is lane-aligned. 24 matmuls total.

The Riccati recursion converges within two chunks, so chunks k >= 2 share one
weight set (verified: output error identical to exact weights). Uploads and
the per-chunk output DMAs are spread across the Sync/Activation HWDGE queues
plus the GpSimd SWDGE queue, since each queue moves only ~90 GB/s serially.
"""

import numpy as np

import concourse.bass as bass
import concourse.mybir as mybir
import concourse.tile as tile
from concourse.bass_utils import run_bass_kernel_spmd

S_DIM = 32
O_DIM = 16
T = 64
CH = 8
NCH = T // CH
B = 2048
NCORES = 8
BS = B // NCORES  # 256
ORDER_B = (4, 5, 7, 6)  # step 7 at rows 64:96 -> lane-aligned carry cast
KSETS = 3  # distinct weight sets: chunk 0, chunk 1, steady state (k >= 2)

F32 = mybir.dt.float32
F16 = mybir.dt.float16


def _host_gains(F, H, Q, R, P0):
    """Batch-uniform Kalman gain/transition sequences, in float64."""
    I = np.eye(S_DIM)
    P = P0
    A_list, K_list = [], []
    for _ in range(T):
        P_pred = F @ P @ F.T + Q
        S = H @ P_pred @ H.T + R
        K = P_pred @ H.T @ np.linalg.inv(S)
        A = (I - K @ H) @ F
        P = (I - K @ H) @ P_pred
        A_list.append(A)
        K_list.append(K)

    G = np.zeros((KSETS, CH, S_DIM, S_DIM))
    L = np.zeros((KSETS, CH, CH, S_DIM, O_DIM))
    for k in range(KSETS):
        for i in range(CH):
            t = CH * k + i
            G[k, i] = A_list[t] @ (G[k, i - 1] if i > 0 else I)
            for j in range(i):
                L[k, i, j] = A_list[t] @ L[k, i - 1, j]
            L[k, i, i] = K_list[t]
    return G, L


def _pack_weights(G, L):
    """wall [96, KSETS, 3, 128]: slot 0 = [la; ga], slot 1 = [lb(j<4); gb]
    (B-tile cols in ORDER_B), slot 2 rows 0:64 = lb(j=4..7)."""
    wall = np.zeros((96, KSETS, 3, 128))
    for k in range(KSETS):
        for ii in range(4):
            wall[64:96, k, 0, ii * 32:(ii + 1) * 32] = G[k, ii].T
            for j in range(ii + 1):
                wall[j * 16:(j + 1) * 16, k, 0, ii * 32:(ii + 1) * 32] = \
                    L[k, ii, j].T
        for idx, i in enumerate(ORDER_B):
            wall[64:96, k, 1, idx * 32:(idx + 1) * 32] = G[k, i].T
            for j in range(min(i + 1, 4)):
                wall[j * 16:(j + 1) * 16, k, 1, idx * 32:(idx + 1) * 32] = \
                    L[k, i, j].T
            for j in range(4, i + 1):
                wall[(j - 4) * 16:(j - 3) * 16, k, 2,
                     idx * 32:(idx + 1) * 32] = L[k, i, j].T
    return wall.astype(np.float16)


def build_nc(split_waits=True):
    nc = bass.Bass("TRN2", target_bir_lowering=False, debug=False,
                   num_devices=NCORES)

    zgz_d = nc.dram_tensor("zgz", (64, NCH, BS), F16, kind="ExternalInput")
    zhi_d = nc.dram_tensor("zhi", (64, NCH, BS), F16, kind="ExternalInput")
    x0t_d = nc.dram_tensor("x0t", (S_DIM, BS), F16, kind="ExternalInput")
    wall_d = nc.dram_tensor("wall", (96, KSETS, 3, 128), F16,
                            kind="ExternalInput")
    out_d = nc.dram_tensor("out", (NCH, 2, 128, BS), F16, kind="ExternalOutput")

    with tile.TileContext(nc) as tc:
        with (
            tc.tile_pool(name="const", bufs=1) as const,
            tc.tile_pool(name="outs", bufs=4) as s_p,
            tc.tile_pool(name="psa", bufs=3, space="PSUM") as ps_a,
            tc.tile_pool(name="psb", bufs=3, space="PSUM") as ps_b,
        ):
            zg = const.tile([96, NCH, BS], F16)   # rows 0:64 z(j<4), 64:96 x
            zhi = const.tile([64, NCH, BS], F16)
            wall = const.tile([96, KSETS, 3, 128], F16)

            nc.scalar.dma_start(wall[:, 0:1, :, :], wall_d[:, 0:1, :, :])
            nc.sync.dma_start(zg[0:64, 0:1, :], zgz_d[:, 0:1, :])
            nc.scalar.dma_start(zg[64:96, 0, :], x0t_d[:])
            nc.sync.dma_start(zhi[:, 0:1, :], zhi_d[:, 0:1, :])
            nc.scalar.dma_start(wall[:, 1:2, :, :], wall_d[:, 1:2, :, :])
            nc.sync.dma_start(zg[0:64, 1:4, :], zgz_d[:, 1:4, :])
            nc.scalar.dma_start(wall[:, 2:3, :, :], wall_d[:, 2:3, :, :])
            nc.sync.dma_start(zhi[:, 1:4, :], zhi_d[:, 1:4, :])
            nc.sync.dma_start(zg[0:64, 4:, :], zgz_d[:, 4:, :])
            nc.sync.dma_start(zhi[:, 4:, :], zhi_d[:, 4:, :])

            # gpsimd early (its SWDGE drain is slow - keep it off the tail),
            # sync takes the late chunks once its input issues are done
            out_eng = {0: nc.gpsimd, 1: nc.scalar, 2: nc.gpsimd, 3: nc.scalar,
                       4: nc.gpsimd, 5: nc.scalar, 6: nc.sync}

            for k in range(NCH):
                ki = min(k, 2)
                last = k == NCH - 1
                zg_k = zg[0:96, k, :]

                # B first: the carry cast off psB overlaps A + next B2,
                # keeping the recurrence off the critical path
                psB = ps_b.tile([128, BS], F32, name="psB")
                nc.tensor.matmul(psB[:], wall[0:64, ki, 2, :],
                                 zhi[0:64, k, :], start=True, stop=False)
                nc.tensor.matmul(psB[:], wall[0:96, ki, 1, :], zg_k,
                                 start=False, stop=True)
                if not last:
                    # carry: step-7 rows land lane-aligned into zg slot k+1
                    nc.scalar.copy(zg[64:96, k + 1, :], psB[64:96, :])
                psA = ps_a.tile([128, BS], F32, name="psA")
                nc.tensor.matmul(psA[:], wall[0:96, ki, 0, :], zg_k,
                                 start=True, stop=True)

                pair = s_p.tile([128, 2, BS], F16, name="pair")
                nc.vector.tensor_copy(pair[:, 1, :], psB[:])
                nc.vector.tensor_copy(pair[:, 0, :], psA[:])
                if not last:
                    out_eng[k].dma_start(
                        out_d[k].rearrange("two p b -> p two b"), pair[:])
                else:
                    nc.sync.dma_start(out_d[k, 0], pair[:, 0, :])
                    nc.scalar.dma_start(out_d[k, 1], pair[:, 1, :])

    if split_waits:
        # the wait-splitting NoOps confuse CoreSim's race detector; the sim
        # path builds without them (identical semantics, redistributed waits)
        _split_matmul_waits(nc)
    return nc


def _split_matmul_waits(nc, max_waits=1):
    """Walrus lowers matmuls/DMAs through templates that support fewer
    sync-wait slots than Tile may emit. Move excess waits onto a NoOp
    inserted right before the offending instruction."""
    for f in nc.m.functions:
        for blk in f.blocks:
            insts = list(blk.instructions)
            out = []
            for inst in insts:
                si = inst.sync_info
                if si is not None and si.on_wait and len(si.on_wait) > max_waits:
                    waits = list(si.on_wait)
                    carry, keep = waits[:-max_waits], waits[-max_waits:]
                    for w in carry:
                        nop = mybir.InstNoOp(
                            name=nc.get_next_instruction_name(),
                            sync_info=mybir.SyncInfo(on_wait=[w], on_update=[]),
                            bass_nofuse=True,
                            engine=inst.engine,
                        )
                        out.append(nop)
                    inst.sync_info = mybir.SyncInfo(
                        on_wait=keep, on_update=list(si.on_update or [])
                    )
                out.append(inst)
            if len(out) != len(insts):
                blk.instructions = out


def _prep_inputs(state0, cov0, measurements, F, H, Q, R):
    """Host-side: gains, packing, measurement pre-transpose. Returns per-core
    input maps."""
    G, L = _host_gains(
        np.asarray(F, np.float64), np.asarray(H, np.float64),
        np.asarray(Q, np.float64), np.asarray(R, np.float64),
        np.asarray(cov0, np.float64)[0],
    )
    wall = _pack_weights(G, L)

    state0 = np.asarray(state0, np.float32)
    measurements = np.asarray(measurements, np.float32)

    in_maps = []
    for c in range(NCORES):
        z = measurements[c * BS:(c + 1) * BS]
        zt = np.ascontiguousarray(
            z.reshape(BS, NCH, CH, O_DIM).transpose(2, 3, 1, 0)
        ).reshape(CH * O_DIM, NCH, BS).astype(np.float16)
        in_maps.append({
            "zgz": np.ascontiguousarray(zt[0:64]),
            "zhi": np.ascontiguousarray(zt[64:128]),
            "x0t": np.ascontiguousarray(
                state0[c * BS:(c + 1) * BS].T).astype(np.float16),
            "wall": wall,
        })
    return in_maps


def _assemble(results):
    """Stitch per-core transposed fp16 outputs into (B, T, S) fp32."""
    out = np.empty((B, T, S_DIM), np.float32)
    for c in range(NCORES):
        arr = np.asarray(results[c]["out"], np.float32).reshape(NCH, 2, 4, 32, BS)
        xA = arr[:, 0].transpose(3, 0, 1, 2)                    # i = 0..3
        xB = arr[:, 1][:, (0, 1, 3, 2)].transpose(3, 0, 1, 2)   # i = 4..7
        out[c * BS:(c + 1) * BS] = np.concatenate(
            [xA, xB], axis=2).reshape(BS, T, S_DIM)
    return out


_CACHE = {}


def kernel(state0, cov0, measurements, F, H, Q, R, _trace=False):
    in_maps = _prep_inputs(state0, cov0, measurements, F, H, Q, R)

    if "nc" not in _CACHE:
        _CACHE["nc"] = build_nc()
    nc = _CACHE["nc"]

    res = run_bass_kernel_spmd(nc, in_maps, core_ids=list(range(NCORES)),
                               trace=_trace)
    out = _assemble(res.results)
    if _trace:
        kernel._last_result = res
    return out
